# revision 16
# baseline (speedup 1.0000x reference)
"""GCN layer (GCNConv on a fully-connected 4096-node graph) on 8 trn2 NeuronCores.

Math (see harness reference):
    A[i, j] = edge_weights[i*4096 + j]          (edge_index is the full meshgrid)
    deg[j]  = sum_i A[i, j]
    d       = deg ** -0.5                        (deg > 0 always here)
    An      = d[:, None] * A * d[None, :]        (symmetric normalization)
    out     = An.T @ (x @ W) + b

Sharding: row-parallel (the sharding hint's alternative): core c owns rows
i in [c*512, (c+1)*512) of An and x. The normalization is folded into An on
the host during the bf16 cast. Each core computes
    h_c = x_c @ W                  (512 x 2048, full W streamed)
    P_c[f, j] = sum_{i in shard} h_c[i, f] * An[i, j]
and the host sums the 8 partials (the "all-reduce" of the hint) and adds b.

Schedule: H and AGG are interleaved in four sections (H f-slab fg, then the
AGG quarter that consumes it) so the W stream spreads over the whole kernel
instead of front-loading; An (4MB) is SBUF-resident; out-partials leave on
the gpsimd SWDGE queue. A few dummy matmuls on a zeroed tile bridge the DMA
head so the PE is HAM-warm when real data lands. fp32 PSUM accumulation.
"""

import sys

sys.path.insert(0, "/opt/trn_rl_repo")

import numpy as np
import ml_dtypes

N = 4096          # nodes
K = 2048          # num_kernels (features)
R = 512           # rows per core (4096 / 8)
RB = R // 128     # 4 row blocks per core
KB = K // 128     # 16 contraction blocks
FG = K // 512     # 4 f-groups of 512
JG = N // 512     # 8 j-groups of 512
P = 128

_BF16 = ml_dtypes.bfloat16
_cache = {}

# kb-ranges for the first x^T / W slab loads: small chunks first so the PE
# can start ~1us after the queue opens, larger ones once it is ahead.
_CHUNKS = [(0, 1), (1, 2), (2, 3), (3, 4), (4, 6), (6, 8), (8, 12), (12, 16)]


def _build():
    import concourse.bass as bass
    import concourse.mybir as mybir
    from concourse import bacc
    from concourse.tile import TileContext

    dt = mybir.dt
    nc = bacc.Bacc("TRN2", target_bir_lowering=False)

    # Ans[jgh, p, ib, j'] = An[c*512 + ib*128 + p, jgh*1024 + j']
    Ans = nc.dram_tensor("Ans", [4, P, RB, 1024], dt.bfloat16, kind="ExternalInput")
    # xTs[p, kb, i] = x[c*512 + i, kb*128 + p]
    xTs = nc.dram_tensor("xTs", [P, KB, R], dt.bfloat16, kind="ExternalInput")
    # Wb[fg, p, kb, f'] = W[kb*128 + p, fg*512 + f']
    Wb = nc.dram_tensor("Wb", [FG, P, KB, 512], dt.bfloat16, kind="ExternalInput")
    # outPb[jg, sg, p, s, j] = P_c[(sg*4+s)*128 + p, jg*512 + j]
    outPb = nc.dram_tensor("outPb", [JG, 4, P, 4, 512], dt.bfloat16,
                           kind="ExternalOutput")

    with TileContext(nc) as tc:
        with (
            tc.tile_pool(name="wz", bufs=1) as wz_pool,
            tc.tile_pool(name="xt", bufs=1) as xt_pool,
            tc.tile_pool(name="an", bufs=4) as an_pool,
            tc.tile_pool(name="w", bufs=2) as w_pool,
            tc.tile_pool(name="z", bufs=1) as z_pool,
            tc.tile_pool(name="st", bufs=3) as st_pool,
            tc.tile_pool(name="ps", bufs=8, space="PSUM") as ps,
        ):
            # x^T shard on sync, need-ordered chunks
            xt = xt_pool.tile([P, KB, R], dt.bfloat16)
            for k0, k1 in _CHUNKS:
                nc.sync.dma_start(
                    out=xt[:, k0:k1, :],
                    in_=bass.AP(
                        tensor=xTs,
                        offset=k0 * R,
                        ap=[[KB * R, P], [R, k1 - k0], [1, R]],
                    ),
                )
            # An shard (j-major chunks), SBUF-resident. jgh0 rides sync right
            # after x^T (needed when AGG0 starts); jgh1-3 are issued on the
            # scalar queue AFTER W fg1 (below) so they do not steal HBM
            # bandwidth from the critical x^T + W fg0 startup streams.
            an_t = [an_pool.tile([P, RB, 1024], dt.bfloat16, name=f"an{jgh}")
                    for jgh in range(4)]

            z_sb = z_pool.tile([P, RB, K], dt.bfloat16)

            # PE warm-up: dummy matmuls on a zeroed tile while the first data
            # chunks are still in flight (results overwritten by the real
            # start=True groups below).
            wz = wz_pool.tile([P, 256], dt.bfloat16)
            nc.vector.memset(wz, 0.0)
            hp = [ps.tile([P, 512], dt.float32, tag="ps", name=f"hp{ib}")
                  for ib in range(RB)]
            for _ in range(2):
                for ib in range(RB):
                    nc.tensor.matmul(hp[ib][:, :256], wz[:, :P], wz, start=True, stop=True)

            for fg in range(FG):
                # ---- H section: h[:, fg-slab] = x_c @ W[:, fg-slab]
                w_t = w_pool.tile([P, KB, 512], dt.bfloat16)
                if fg == 0:
                    for k0, k1 in _CHUNKS:
                        nc.scalar.dma_start(
                            out=w_t[:, k0:k1, :],
                            in_=bass.AP(
                                tensor=Wb,
                                offset=k0 * 512,
                                ap=[[KB * 512, P], [512, k1 - k0], [1, 512]],
                            ),
                        )
                    # All An chunks queue on scalar behind W fg0: they arrive
                    # during AGG0's window without competing with the
                    # critical x^T + W fg0 startup streams (sync carries only
                    # x^T until the final out-stores)
                    for jgh in range(4):
                        nc.scalar.dma_start(out=an_t[jgh], in_=Ans[jgh])
                else:
                    nc.scalar.dma_start(out=w_t, in_=Wb[fg])
                if fg > 0:
                    hp = [ps.tile([P, 512], dt.float32, tag="ps",
                                  name=f"hp{fg}_{ib}") for ib in range(RB)]
                for kb in range(KB):
                    for ib in range(RB):
                        nc.tensor.matmul(
                            hp[ib],
                            xt[:, kb, ib * P:(ib + 1) * P],
                            w_t[:, kb, :],
                            start=(kb == 0),
                            stop=(kb == KB - 1),
                        )
                for ib in range(RB):
                    nc.vector.tensor_copy(z_sb[:, ib, fg * 512:(fg + 1) * 512],
                                          hp[ib])

                # ---- AGG section for fh in [4*fg, 4*fg+4):
                # P_c[fh-block, :] = sum_i z[i, fh-block] An[i, :]
                # j-groups processed in pairs sharing each stationary
                # LDWEIGHTS (z block loads once, streams both j-halves).
                sg = fg
                for u in range(JG // 2):
                    a = an_t[u]
                    stg = [st_pool.tile([P, 4, 512], dt.bfloat16, tag="st",
                                        name=f"stg{fg}_{u}_{v}")
                           for v in range(2)]
                    for s in range(4):
                        fh = sg * 4 + s
                        op = [ps.tile([P, 512], dt.float32, tag="ps",
                                      name=f"op{fg}_{u}_{s}_{v}")
                              for v in range(2)]
                        for ib in range(RB):
                            for v in range(2):
                                nc.tensor.matmul(
                                    op[v],
                                    z_sb[:, ib, fh * P:(fh + 1) * P],
                                    a[:, ib, v * 512:(v + 1) * 512],
                                    start=(ib == 0),
                                    stop=(ib == RB - 1),
                                )
                        for v in range(2):
                            nc.vector.tensor_copy(stg[v][:, s, :], op[v])
                    for v in range(2):
                        jg = 2 * u + v
                        # last section's stores ride the (by then idle) HWDGE
                        # sync queue; the very last one in two halves for a
                        # shorter tail
                        if fg == FG - 1:
                            if u == JG // 2 - 1 and v == 1:
                                base = (jg * 4 + sg) * P * 4 * 512
                                for half in range(2):
                                    nc.sync.dma_start(
                                        out=bass.AP(
                                            tensor=outPb,
                                            offset=base + half * 1024,
                                            ap=[[4 * 512, P], [512, 2], [1, 512]],
                                        ),
                                        in_=stg[v][:, 2 * half:2 * half + 2, :],
                                    )
                            else:
                                nc.sync.dma_start(out=outPb[jg, sg], in_=stg[v])
                        else:
                            nc.gpsimd.dma_start(out=outPb[jg, sg], in_=stg[v])

    nc.compile()
    return nc


def _get_nc():
    if "nc" not in _cache:
        _cache["nc"] = _build()
    return _cache["nc"]


def _prep_inputs(x, edge_weights, W, b):
    A32 = np.asarray(edge_weights, np.float32).reshape(N, N)
    deg = A32.sum(axis=0, dtype=np.float64)
    d = 1.0 / np.sqrt(deg)
    An = (A32 * d[None, :].astype(np.float32)) * d[:, None].astype(np.float32)
    An16 = An.astype(_BF16)
    x16 = np.asarray(x, np.float32).astype(_BF16)
    W16 = np.asarray(W, np.float32).astype(_BF16)
    # Wb[fg, p, kb, f'] = W[kb*128+p, fg*512+f']  (shared by all cores)
    Wb = np.ascontiguousarray(
        W16.reshape(KB, P, FG, 512).transpose(2, 1, 0, 3)
    )
    in_maps = []
    for c in range(8):
        rows = slice(c * R, (c + 1) * R)
        # Ans[jgh, p, ib, j'] = An[c*512 + ib*128 + p, jgh*1024 + j']
        Ans = np.ascontiguousarray(
            An16[rows].reshape(RB, P, 4, 1024).transpose(2, 1, 0, 3)
        )
        # xTs[p, kb, i] = x[c*512 + i, kb*128 + p]
        xTs = np.ascontiguousarray(
            x16[rows].reshape(R, KB, P).transpose(2, 1, 0)
        )
        in_maps.append({"Ans": Ans, "xTs": xTs, "Wb": Wb})
    return in_maps


def _run(in_maps, trace=False):
    from concourse.bass_utils import run_bass_kernel_spmd

    nc = _get_nc()
    return run_bass_kernel_spmd(nc, in_maps, list(range(8)), trace=trace)


def kernel(x, edge_index, edge_weights, W, b):
    in_maps = _prep_inputs(x, edge_weights, W, b)
    res = _run(in_maps)
    # host-side all-reduce of the 8 row-shard partials
    acc = np.zeros((K, N), np.float32)
    for c in range(8):
        # outPb [8, 4, 128, 4, 512] -> P_c [2048, 4096]
        Pc = (
            np.asarray(res.results[c]["outPb"])
            .transpose(1, 3, 2, 0, 4)
            .reshape(K, N)
            .astype(np.float32)
        )
        acc += Pc
    out = acc.T + np.asarray(b, np.float32)[None, :]
    return np.ascontiguousarray(out)


# revision 19
# speedup vs baseline: 1.0645x; 1.0645x over previous
"""GCN layer (GCNConv on a fully-connected 4096-node graph) on 8 trn2 NeuronCores.

Math (see harness reference):
    A[i, j] = edge_weights[i*4096 + j]          (edge_index is the full meshgrid)
    deg[j]  = sum_i A[i, j]
    d       = deg ** -0.5                        (deg > 0 always here)
    An      = d[:, None] * A * d[None, :]        (symmetric normalization)
    out     = An.T @ (x @ W) + b

Sharding: row-parallel (the sharding hint's alternative): core c owns rows
i in [c*512, (c+1)*512) of An and x. The normalization is folded into An on
the host during the bf16 cast. Each core computes
    h_c = x_c @ W                  (512 x 2048, full W streamed)
    P_c[f, j] = sum_{i in shard} h_c[i, f] * An[i, j]
and the host sums the 8 partials (the "all-reduce" of the hint) and adds b.

Schedule: H and AGG are interleaved in four sections (H f-slab fg, then the
AGG quarter that consumes it) so the W stream spreads over the whole kernel
instead of front-loading; An (4MB) is SBUF-resident; out-partials leave on
the gpsimd SWDGE queue. A few dummy matmuls on a zeroed tile bridge the DMA
head so the PE is HAM-warm when real data lands. fp32 PSUM accumulation.
"""

import sys

sys.path.insert(0, "/opt/trn_rl_repo")

import numpy as np
import ml_dtypes

N = 4096          # nodes
K = 2048          # num_kernels (features)
R = 512           # rows per core (4096 / 8)
RB = R // 128     # 4 row blocks per core
KB = K // 128     # 16 contraction blocks
FG = K // 512     # 4 f-groups of 512
JG = N // 512     # 8 j-groups of 512
P = 128

_BF16 = ml_dtypes.bfloat16
_cache = {}

# kb-ranges for the first x^T / W slab loads: small chunks first so the PE
# can start ~1us after the queue opens, larger ones once it is ahead.
_CHUNKS = [(0, 1), (1, 2), (2, 3), (3, 4), (4, 6), (6, 8), (8, 12), (12, 16)]


def _build():
    import concourse.bass as bass
    import concourse.mybir as mybir
    from concourse import bacc
    from concourse.tile import TileContext

    dt = mybir.dt
    nc = bacc.Bacc("TRN2", target_bir_lowering=False)

    # Ans[jgh, p, ib, j'] = An[c*512 + ib*128 + p, jgh*1024 + j']
    Ans = nc.dram_tensor("Ans", [4, P, RB, 1024], dt.bfloat16, kind="ExternalInput")
    # xTs[p, kb, i] = x[c*512 + i, kb*128 + p]
    xTs = nc.dram_tensor("xTs", [P, KB, R], dt.bfloat16, kind="ExternalInput")
    # Wb[fg, p, kb, f'] = W[kb*128 + p, fg*512 + f']
    Wb = nc.dram_tensor("Wb", [FG, P, KB, 512], dt.bfloat16, kind="ExternalInput")
    # outPb[jg, sg, p, s, j] = P_c[(sg*4+s)*128 + p, jg*512 + j]
    outPb = nc.dram_tensor("outPb", [JG, 4, P, 4, 512], dt.bfloat16,
                           kind="ExternalOutput")

    with TileContext(nc) as tc:
        with (
            tc.tile_pool(name="wz", bufs=1) as wz_pool,
            tc.tile_pool(name="xt", bufs=1) as xt_pool,
            tc.tile_pool(name="an", bufs=4) as an_pool,
            tc.tile_pool(name="w", bufs=2) as w_pool,
            tc.tile_pool(name="z", bufs=1) as z_pool,
            tc.tile_pool(name="st", bufs=3) as st_pool,
            tc.tile_pool(name="ps", bufs=8, space="PSUM") as ps,
        ):
            # x^T shard on sync, need-ordered chunks
            xt = xt_pool.tile([P, KB, R], dt.bfloat16)
            for k0, k1 in _CHUNKS:
                nc.sync.dma_start(
                    out=xt[:, k0:k1, :],
                    in_=bass.AP(
                        tensor=xTs,
                        offset=k0 * R,
                        ap=[[KB * R, P], [R, k1 - k0], [1, R]],
                    ),
                )
            # An shard (j-major chunks), SBUF-resident. an0/an1 ride sync
            # right after x^T (that queue is idle once x^T lands); an2/an3
            # ride scalar behind W fg0 — so neither queue competes with the
            # critical x^T + W fg0 startup streams and the four chunks land
            # well before their AGG0 j-groups need them.
            an_t = [an_pool.tile([P, RB, 1024], dt.bfloat16, name=f"an{jgh}")
                    for jgh in range(4)]
            for jgh in range(2):
                nc.sync.dma_start(out=an_t[jgh], in_=Ans[jgh])

            z_sb = z_pool.tile([P, RB, K], dt.bfloat16)

            # PE warm-up: dummy matmuls on a zeroed tile while the first data
            # chunks are still in flight (results overwritten by the real
            # start=True groups below).
            wz = wz_pool.tile([P, 256], dt.bfloat16)
            nc.vector.memset(wz, 0.0)
            hp = [ps.tile([P, 512], dt.float32, tag="ps", name=f"hp{ib}")
                  for ib in range(RB)]
            for _ in range(2):
                for ib in range(RB):
                    nc.tensor.matmul(hp[ib][:, :256], wz[:, :P], wz, start=True, stop=True)

            for fg in range(FG):
                # ---- H section: h[:, fg-slab] = x_c @ W[:, fg-slab]
                w_t = w_pool.tile([P, KB, 512], dt.bfloat16)
                if fg == 0:
                    for k0, k1 in _CHUNKS:
                        nc.scalar.dma_start(
                            out=w_t[:, k0:k1, :],
                            in_=bass.AP(
                                tensor=Wb,
                                offset=k0 * 512,
                                ap=[[KB * 512, P], [512, k1 - k0], [1, 512]],
                            ),
                        )
                    for jgh in range(2, 4):
                        nc.scalar.dma_start(out=an_t[jgh], in_=Ans[jgh])
                else:
                    nc.scalar.dma_start(out=w_t, in_=Wb[fg])
                if fg > 0:
                    hp = [ps.tile([P, 512], dt.float32, tag="ps",
                                  name=f"hp{fg}_{ib}") for ib in range(RB)]
                for kb in range(KB):
                    for ib in range(RB):
                        nc.tensor.matmul(
                            hp[ib],
                            xt[:, kb, ib * P:(ib + 1) * P],
                            w_t[:, kb, :],
                            start=(kb == 0),
                            stop=(kb == KB - 1),
                        )
                for ib in range(RB):
                    nc.vector.tensor_copy(z_sb[:, ib, fg * 512:(fg + 1) * 512],
                                          hp[ib])

                # ---- AGG section for fh in [4*fg, 4*fg+4):
                # P_c[fh-block, :] = sum_i z[i, fh-block] An[i, :]
                sg = fg
                for jg in range(JG):
                    stage = st_pool.tile([P, 4, 512], dt.bfloat16, tag="st",
                                         name=f"stg{fg}_{jg}")
                    a = an_t[jg // 2]
                    jh = (jg % 2) * 512
                    for s in range(4):
                        fh = sg * 4 + s
                        op = ps.tile([P, 512], dt.float32, tag="ps")
                        for ib in range(RB):
                            nc.tensor.matmul(
                                op,
                                z_sb[:, ib, fh * P:(fh + 1) * P],
                                a[:, ib, jh:jh + 512],
                                start=(ib == 0),
                                stop=(ib == RB - 1),
                            )
                        nc.vector.tensor_copy(stage[:, s, :], op)
                    # last section's stores ride the (by then idle) HWDGE
                    # sync queue; the very last one in two halves for a
                    # shorter tail
                    if fg == FG - 1:
                        if jg == JG - 1:
                            base = (jg * 4 + sg) * P * 4 * 512
                            for half in range(2):
                                nc.sync.dma_start(
                                    out=bass.AP(
                                        tensor=outPb,
                                        offset=base + half * 1024,
                                        ap=[[4 * 512, P], [512, 2], [1, 512]],
                                    ),
                                    in_=stage[:, 2 * half:2 * half + 2, :],
                                )
                        else:
                            nc.sync.dma_start(out=outPb[jg, sg], in_=stage)
                    else:
                        nc.gpsimd.dma_start(out=outPb[jg, sg], in_=stage)

    nc.compile()
    return nc


def _get_nc():
    if "nc" not in _cache:
        _cache["nc"] = _build()
    return _cache["nc"]


def _prep_inputs(x, edge_weights, W, b):
    A32 = np.asarray(edge_weights, np.float32).reshape(N, N)
    deg = A32.sum(axis=0, dtype=np.float64)
    d = 1.0 / np.sqrt(deg)
    An = (A32 * d[None, :].astype(np.float32)) * d[:, None].astype(np.float32)
    An16 = An.astype(_BF16)
    x16 = np.asarray(x, np.float32).astype(_BF16)
    W16 = np.asarray(W, np.float32).astype(_BF16)
    # Wb[fg, p, kb, f'] = W[kb*128+p, fg*512+f']  (shared by all cores)
    Wb = np.ascontiguousarray(
        W16.reshape(KB, P, FG, 512).transpose(2, 1, 0, 3)
    )
    in_maps = []
    for c in range(8):
        rows = slice(c * R, (c + 1) * R)
        # Ans[jgh, p, ib, j'] = An[c*512 + ib*128 + p, jgh*1024 + j']
        Ans = np.ascontiguousarray(
            An16[rows].reshape(RB, P, 4, 1024).transpose(2, 1, 0, 3)
        )
        # xTs[p, kb, i] = x[c*512 + i, kb*128 + p]
        xTs = np.ascontiguousarray(
            x16[rows].reshape(R, KB, P).transpose(2, 1, 0)
        )
        in_maps.append({"Ans": Ans, "xTs": xTs, "Wb": Wb})
    return in_maps


def _run(in_maps, trace=False):
    from concourse.bass_utils import run_bass_kernel_spmd

    nc = _get_nc()
    return run_bass_kernel_spmd(nc, in_maps, list(range(8)), trace=trace)


def kernel(x, edge_index, edge_weights, W, b):
    in_maps = _prep_inputs(x, edge_weights, W, b)
    res = _run(in_maps)
    # host-side all-reduce of the 8 row-shard partials
    acc = np.zeros((K, N), np.float32)
    for c in range(8):
        # outPb [8, 4, 128, 4, 512] -> P_c [2048, 4096]
        Pc = (
            np.asarray(res.results[c]["outPb"])
            .transpose(1, 3, 2, 0, 4)
            .reshape(K, N)
            .astype(np.float32)
        )
        acc += Pc
    out = acc.T + np.asarray(b, np.float32)[None, :]
    return np.ascontiguousarray(out)
